# revision 6
# baseline (speedup 1.0000x reference)
"""Self-contained Trainium2 Bass kernel for the bidirectional-LSTM decoder
(nn_Decoder): 1-layer 2-direction LSTM scan over T=100 steps with a fixed
input, followed by a 32000-way vocab projection and log_softmax, distributed
over 8 NeuronCores.

Distribution (single fused SPMD launch):
  - Scan: direction-split. Cores 0-3 compute the forward LSTM for all 64
    batch rows, cores 4-7 the backward one. Weights ship as fp8e4m3
    quarters and are reassembled on-device with AllGather; fp8 stationary
    weights halve the LDWEIGHTS bandwidth that bounds the scan.
  - Transpose-free scan layout: gates^T[4H, B] accumulated per gate-quarter
    (i/f/g/o) into its own PSUM bank so the nonlinearity of quarter q
    pipelines under the matmuls of quarter q+1.
  - LSTM outputs exchange: chunked pair-AllGather (4 chunks of T/4 steps)
    overlaps the remaining scan.
  - fc + log_softmax: vocab-split (4000 rows/core). ONE matmul sweep in
    fp8 DoubleRow (K=256 per pass; o_t and fc_W both fp8), logits+bias
    staged to DRAM in bf16, softmax normalizer AllReduced in two halves so
    the fixup of half 0 (DMA+DVE: read logits, add -logZ, write output)
    overlaps the matmul sweep of half 1.
  - All recurrent state stays fp32/bf16; end-to-end max relative error vs
    the fp32 reference ~2e-3.
"""

import os
import sys

if "/opt/trn_rl_repo" not in sys.path:
    sys.path.insert(0, "/opt/trn_rl_repo")

from contextlib import ExitStack

import numpy as np

import concourse.bass as bass
import concourse.tile as tile
from concourse import mybir
from concourse.bass_utils import run_bass_kernel_spmd

F32 = mybir.dt.float32
BF16 = mybir.dt.bfloat16
FP8 = mybir.dt.float8e4
NP_BF16 = mybir.dt.np(BF16)
NP_FP8 = mybir.dt.np(FP8)
B = 64
H = 1024
V = 32000
NCORES = 8

W_SCALE = 256.0  # fp8 quant scale for W_hh / W_ih
O_SCALE = 64.0  # fp8 quant scale for LSTM outputs entering fc
FCW_SCALE = 2048.0  # fp8 quant scale for fc_W
LG_DESCALE = 1.0 / (O_SCALE * FCW_SCALE)

MAX_WAITS = 1

USE_DR = os.environ.get("BK_DR", "1") == "1"  # fp8 DoubleRow on fc matmul
N_AG_CHUNKS = int(os.environ.get("BK_AGC", "4"))  # outs AllGather chunks
N_STAT_HALVES = int(os.environ.get("BK_SH", "2"))  # stats AllReduce splits
PHASE = os.environ.get("BK_PHASE", "all")  # all | scan | fc  (timing builds only)


def split_multiwait(nc):
    """The walrus build in this environment rejects any instruction carrying
    more than one semaphore wait; hoist excess waits onto chained NOPs
    (sem-ge waits commute, so this preserves semantics)."""
    import bass_rust

    n_split = 0
    for f in nc.m.functions:
        for bb in f.blocks:
            new_insts = []
            changed = False
            for ins in bb.instructions:
                si = ins.sync_info
                if si is not None and si.on_wait and len(si.on_wait) > MAX_WAITS:
                    waits = list(si.on_wait)
                    extra, keep = waits[:-MAX_WAITS], waits[-MAX_WAITS:]
                    for j in range(0, len(extra), MAX_WAITS):
                        nop = bass_rust.InstNoOp(name=f"{ins.name}-wsplit{j}")
                        nop.engine = ins.engine
                        nop.sync_info = mybir.SyncInfo(
                            on_wait=extra[j : j + MAX_WAITS], on_update=[]
                        )
                        new_insts.append(nop)
                        n_split += 1
                    ins.sync_info = mybir.SyncInfo(
                        on_wait=keep, on_update=list(si.on_update)
                    )
                    changed = True
                new_insts.append(ins)
            if changed:
                bb.instructions = new_insts
    return n_split


def build_fused(T, v_loc=V // NCORES, timing=False):
    n_tok = B * T
    n_tt = n_tok // 128
    n_vc = v_loc // 500
    assert n_tok % 128 == 0 and v_loc % 500 == 0 and T % 2 == 0
    n_agc = N_AG_CHUNKS if T % (2 * N_AG_CHUNKS) == 0 else 1
    t_ch = T // n_agc  # steps per AllGather chunk
    n_sh = N_STAT_HALVES if n_tt % N_STAT_HALVES == 0 else 1
    tt_h = n_tt // n_sh  # token tiles per stats group

    nc = bass.Bass(num_devices=NCORES)
    whh_q = nc.declare_dram_parameter("whh_q", [2, 128, 32, 128], FP8, isOutput=False)
    wih_q = nc.declare_dram_parameter("wih_q", [2, 128, 32, 128], FP8, isOutput=False)
    xT = nc.declare_dram_parameter("xT", [128, 8, 64], BF16, isOutput=False)
    h0T = nc.declare_dram_parameter("h0T", [128, 8, 64], BF16, isOutput=False)
    c0T = nc.declare_dram_parameter("c0T", [128, 8, 64], F32, isOutput=False)
    biasT = nc.declare_dram_parameter("biasT", [128, 32], F32, isOutput=False)
    fcwT = nc.declare_dram_parameter("fcwT", [128, 16, v_loc], FP8, isOutput=False)
    fcb = nc.declare_dram_parameter("fcb", [1, v_loc], F32, isOutput=False)
    if timing:
        out = nc.dram_tensor("out", [n_tt, 128, v_loc], F32)
        chk = nc.declare_dram_parameter("chk", [128, 64], F32, isOutput=True)
    else:
        out = nc.declare_dram_parameter("out", [n_tt, 128, v_loc], F32, isOutput=True)

    ci_whh = nc.dram_tensor("ci_whh", [2, 128, 32, 128], FP8)
    co_whh = nc.dram_tensor("co_whh", [8, 128, 32, 128], FP8)
    ci_wih = nc.dram_tensor("ci_wih", [2, 128, 32, 128], FP8)
    co_wih = nc.dram_tensor("co_wih", [8, 128, 32, 128], FP8)
    outs_nat = nc.dram_tensor("outs_nat", [T, 128, 512], BF16)
    # chunk k holds [fwd steps k*t_ch..(k+1)*t_ch) ; bwd same steps]
    outs_ga = nc.dram_tensor("outs_ga", [n_agc, 2 * t_ch, 128, 512], BF16)
    lgs = nc.dram_tensor("lgs", [n_tt, 128, v_loc], BF16)
    cc_s_in = nc.dram_tensor("cc_s_in", [n_sh, 128, tt_h], F32)
    cc_s_out = nc.dram_tensor("cc_s_out", [n_sh, 128, tt_h], F32)

    ACT = mybir.ActivationFunctionType
    ALU = mybir.AluOpType
    DIR_GROUPS = [[0, 1, 2, 3], [4, 5, 6, 7]]
    PAIR_GROUPS = [[0, 4], [1, 5], [2, 6], [3, 7]]
    ALL_GROUP = [list(range(NCORES))]

    with tile.TileContext(nc) as tc, ExitStack() as ctx:
        # ---- weight gather (params -> internal -> AllGather) ----
        nc.sync.dma_start(out=ci_wih[:], in_=wih_q[:])
        nc.sync.dma_start(out=ci_whh[:], in_=whh_q[:])
        nc.gpsimd.collective_compute(
            "AllGather", ALU.bypass, replica_groups=DIR_GROUPS,
            ins=[ci_wih[:]], outs=[co_wih[:]],
        )
        nc.gpsimd.collective_compute(
            "AllGather", ALU.bypass, replica_groups=DIR_GROUPS,
            ins=[ci_whh[:]], outs=[co_whh[:]],
        )

        smalls = ctx.enter_context(tc.tile_pool(name="smalls", bufs=1))
        bias_sb = smalls.tile([128, 32], F32)
        nc.sync.dma_start(out=bias_sb, in_=biasT[:])
        h_sb = smalls.tile([128, 8, 64], BF16)
        nc.sync.dma_start(out=h_sb, in_=h0T[:])
        c_sb = smalls.tile([128, 8, 64], F32)
        nc.sync.dma_start(out=c_sb, in_=c0T[:])
        gx_sb = smalls.tile([128, 32, 64], F32)

        # fc weights/bias: loaded up-front so the DMA hides under the scan
        fc_sing = ctx.enter_context(tc.tile_pool(name="fc_sing", bufs=1))
        w_sb = fc_sing.tile([128, 16, v_loc], FP8)
        nc.sync.dma_start(out=w_sb, in_=fcwT[:])
        bias_bc = fc_sing.tile([128, v_loc], F32)
        fcb_ap = fcb[:]
        nc.sync.dma_start(
            out=bias_bc,
            in_=bass.AP(
                tensor=fcb_ap.tensor, offset=fcb_ap.offset, ap=[[0, 128], [1, v_loc]]
            ),
        )

        # ---- scan phase (scoped pools so fc SBUF fits afterwards) ----
        with (
            tc.tile_pool(name="whh_pool", bufs=1) as whh_pool,
            tc.tile_pool(name="scan_work", bufs=2) as work,
        ):
            whh_sb = whh_pool.tile([128, 8, 32, 128], FP8)
            for j in range(8):
                nc.sync.dma_start(out=whh_sb[:, j, :, :], in_=co_whh[j])

            with (
                tc.tile_pool(name="wih_pool", bufs=1) as wih_pool,
                tc.tile_pool(name="psum_gx", bufs=4, space="PSUM") as psum_gx,
            ):
                x_sb = smalls.tile([128, 8, 64], BF16)
                nc.sync.dma_start(out=x_sb, in_=xT[:])
                wih_sb = wih_pool.tile([128, 8, 32, 128], FP8)
                for j in range(8):
                    nc.sync.dma_start(out=wih_sb[:, j, :, :], in_=co_wih[j])
                for m in range(32):
                    pg = psum_gx.tile([128, 64], F32)
                    for j in range(8):
                        nc.tensor.matmul(
                            pg, wih_sb[:, j, m, :], x_sb[:, j, :],
                            start=(j == 0), stop=(j == 7),
                        )
                    nc.vector.tensor_scalar(
                        gx_sb[:, m, :], pg, 1.0 / W_SCALE, bias_sb[:, m : m + 1],
                        op0=ALU.mult, op1=ALU.add,
                    )

            with tc.tile_pool(name="psum_scan", bufs=2, space="PSUM") as psum_main:
                for t in range(T):
                    nl = work.tile([128, 32, 64], F32, tag="nl")
                    for q in range(4):  # gate quarters i, f, g, o
                        pg = psum_main.tile([128, 8, 64], F32, tag=f"pg{q}")
                        for ml in range(8):
                            m = q * 8 + ml
                            for j in range(8):
                                nc.tensor.matmul(
                                    pg[:, ml, :], whh_sb[:, j, m, :], h_sb[:, j, :],
                                    start=(j == 0), stop=(j == 7),
                                )
                        gq = work.tile([128, 8, 64], F32, tag=f"g{q}")
                        nc.vector.scalar_tensor_tensor(
                            gq, pg, 1.0 / W_SCALE, gx_sb[:, q * 8 : q * 8 + 8, :],
                            op0=ALU.mult, op1=ALU.add,
                        )
                        nc.scalar.activation(
                            nl[:, q * 8 : q * 8 + 8, :], gq,
                            ACT.Tanh if q == 2 else ACT.Sigmoid,
                        )
                    t1 = work.tile([128, 8, 64], F32, tag="t1")
                    nc.vector.tensor_mul(t1, nl[:, 0:8, :], nl[:, 16:24, :])
                    t2 = work.tile([128, 8, 64], F32, tag="t2")
                    nc.vector.tensor_mul(t2, nl[:, 8:16, :], c_sb)
                    nc.vector.tensor_add(c_sb, t1, t2)
                    tanh_c = work.tile([128, 8, 64], F32, tag="tanh_c")
                    nc.scalar.activation(tanh_c, c_sb, ACT.Tanh)
                    nc.vector.tensor_mul(h_sb, nl[:, 24:32, :], tanh_c)
                    nc.sync.dma_start(
                        out=outs_nat[t], in_=h_sb.rearrange("p j b -> p (j b)")
                    )
                    if (t + 1) % t_ch == 0:
                        k = (t + 1) // t_ch - 1
                        nc.gpsimd.collective_compute(
                            "AllGather", ALU.bypass, replica_groups=PAIR_GROUPS,
                            ins=[outs_nat[k * t_ch : (t + 1)]], outs=[outs_ga[k]],
                        )

        # ---- fc: ONE fp8-DR matmul sweep + bf16 logit staging + fixup ----
        slab_pool = ctx.enter_context(tc.tile_pool(name="slabs", bufs=2))
        psum_fc = ctx.enter_context(tc.tile_pool(name="psum_fc", bufs=1, space="PSUM"))
        wk = ctx.enter_context(tc.tile_pool(name="fc_work", bufs=2))
        lg_pool = ctx.enter_context(tc.tile_pool(name="fc_lg", bufs=2))
        fix_pool = ctx.enter_context(tc.tile_pool(name="fc_fix", bufs=2))
        spool = ctx.enter_context(tc.tile_pool(name="fc_stats", bufs=1))

        s_all = spool.tile([128, n_tt], F32)
        nc.vector.memset(s_all, 0.0)
        nlz = spool.tile([128, n_tt], F32)

        def pass0_tile(tt):
            o_t = slab_pool.tile([128, 16, 128], FP8 if USE_DR else BF16, tag="o_t")
            for d in range(2):
                sl = slab_pool.tile([128, 2, 512], BF16, tag=f"sl{d}")
                for t2 in range(2):
                    t = 2 * tt + t2
                    ch, ix = divmod(t, t_ch)
                    nc.sync.dma_start(
                        out=sl[:, t2, :], in_=outs_ga[ch][d * t_ch + ix]
                    )
                dst = o_t[:, d * 8 : d * 8 + 8, :].rearrange(
                    "p j (t b) -> p j t b", t=2
                )
                src = sl.rearrange("p t (j b) -> p j t b", j=8)
                if USE_DR:
                    nc.vector.tensor_scalar_mul(dst, src, O_SCALE)
                else:
                    nc.vector.tensor_copy(dst, src)
            lg = lg_pool.tile([128, v_loc], BF16, tag="lg")
            pss = [
                psum_fc.tile([128, 500], F32, tag=f"ps{c}", name=f"ps{c}")
                for c in range(n_vc)
            ]
            n_kp = 8 if USE_DR else 16
            for p in range(n_kp):
                for c in range(n_vc):
                    vs = slice(c * 500, (c + 1) * 500)
                    if USE_DR:
                        nc.tensor.matmul(
                            pss[c], o_t[:, 2 * p : 2 * p + 2, :],
                            w_sb[:, 2 * p : 2 * p + 2, vs],
                            start=(p == 0), stop=(p == n_kp - 1),
                            perf_mode=mybir.MatmulPerfMode.DoubleRow,
                        )
                    else:
                        nc.tensor.matmul(
                            pss[c], o_t[:, p, :], w_sb[:, p, vs],
                            start=(p == 0), stop=(p == n_kp - 1),
                        )
            for c in range(n_vc):
                vs = slice(c * 500, (c + 1) * 500)
                scale = LG_DESCALE if USE_DR else (1.0 / FCW_SCALE)
                nc.vector.scalar_tensor_tensor(
                    lg[:, vs], pss[c], scale, bias_bc[:, vs],
                    op0=ALU.mult, op1=ALU.add,
                )
                ex = wk.tile([128, 500], F32, tag="ex")
                part = wk.tile([128, 1], F32, tag="part")
                nc.scalar.activation(ex, lg[:, vs], ACT.Exp, accum_out=part)
                nc.vector.tensor_add(
                    s_all[:, tt : tt + 1], s_all[:, tt : tt + 1], part
                )
            nc.sync.dma_start(out=lgs[tt], in_=lg)

        def stats_group(h):
            ts = slice(h * tt_h, (h + 1) * tt_h)
            nc.gpsimd.dma_start(out=cc_s_in[h], in_=s_all[:, ts])
            nc.gpsimd.collective_compute(
                "AllReduce", ALU.add, replica_groups=ALL_GROUP,
                ins=[cc_s_in[h]], outs=[cc_s_out[h]],
            )
            s_glob = spool.tile([128, tt_h], F32, tag=f"sg{h}")
            nc.gpsimd.dma_start(out=s_glob, in_=cc_s_out[h])
            lnv = spool.tile([128, tt_h], F32, tag=f"ln{h}")
            nc.scalar.activation(lnv, s_glob, ACT.Ln)
            nc.vector.tensor_scalar_mul(nlz[:, ts], lnv, -1.0)

        def fixup_tile(tt):
            lg_in = fix_pool.tile([128, v_loc], BF16, tag="lgin")
            nc.sync.dma_start(out=lg_in, in_=lgs[tt])
            outt = fix_pool.tile([128, v_loc], F32, tag="outt")
            nc.vector.tensor_scalar_add(outt, lg_in, nlz[:, tt : tt + 1])
            nc.sync.dma_start(out=out[tt], in_=outt)

        if not (timing and PHASE == "scan"):
            for h in range(n_sh):
                for tt in range(h * tt_h, (h + 1) * tt_h):
                    pass0_tile(tt)
                stats_group(h)
            for h in range(n_sh):
                for tt in range(h * tt_h, (h + 1) * tt_h):
                    fixup_tile(tt)

        if timing:
            chk_sb = spool.tile([128, 64], F32)
            if PHASE == "scan":
                nc.vector.tensor_copy(
                    chk_sb, h_sb.rearrange("p j b -> p (j b)")[:, :64]
                )
            else:
                nc.vector.tensor_copy(chk_sb[:, :n_tt], nlz)
            nc.sync.dma_start(out=chk[:, :n_tt], in_=chk_sb[:, :n_tt])

    split_multiwait(nc)
    return nc


def prep_fused_inputs(x, h0, c0, W_ih, W_hh, b_ih, b_hh, fc_W, fc_b, T):
    """Per-core in_maps. Core c: direction d = c//4, weight quarter qc = c%4,
    vocab slice c. Weights quantize to fp8e4m3 with power-of-2 scales."""
    v_loc = V // NCORES
    maps = []
    per_dir = {}
    for d in (0, 1):
        whh_full = (W_hh[d] * W_SCALE).astype(NP_FP8)
        wih_full = (W_ih[d] * W_SCALE).astype(NP_FP8)
        whh_full = whh_full.reshape(32, 128, 8, 128).transpose(3, 2, 0, 1)  # [p,j,m,q]
        wih_full = wih_full.reshape(32, 128, 8, 128).transpose(3, 2, 0, 1)
        per_dir[d] = {
            "whh": whh_full,
            "wih": wih_full,
            "h0T": np.ascontiguousarray(
                h0[d].reshape(64, 8, 128).transpose(2, 1, 0)
            ).astype(NP_BF16),
            "c0T": np.ascontiguousarray(
                c0[d].reshape(64, 8, 128).transpose(2, 1, 0)
            ).astype(np.float32),
            "biasT": np.ascontiguousarray(
                (b_ih[d] + b_hh[d]).reshape(32, 128).T
            ).astype(np.float32),
        }
    xT = np.ascontiguousarray(x.reshape(64, 8, 128).transpose(2, 1, 0)).astype(
        NP_BF16
    )
    for c in range(NCORES):
        d, qc = c // 4, c % 4
        pd = per_dir[d]
        whh_q = np.ascontiguousarray(
            pd["whh"][:, 2 * qc : 2 * qc + 2].transpose(1, 0, 2, 3)
        )
        wih_q = np.ascontiguousarray(
            pd["wih"][:, 2 * qc : 2 * qc + 2].transpose(1, 0, 2, 3)
        )
        wv = (fc_W[c * v_loc : (c + 1) * v_loc] * FCW_SCALE).astype(NP_FP8)
        fcwT = np.ascontiguousarray(wv.reshape(v_loc, 16, 128).transpose(2, 1, 0))
        maps.append(
            {
                "whh_q": whh_q,
                "wih_q": wih_q,
                "xT": xT,
                "h0T": pd["h0T"],
                "c0T": pd["c0T"],
                "biasT": pd["biasT"],
                "fcwT": fcwT,
                "fcb": np.ascontiguousarray(
                    fc_b[c * v_loc : (c + 1) * v_loc].reshape(1, v_loc)
                ).astype(np.float32),
            }
        )
    return maps


def assemble_output(results, T):
    """results[c]["out"] is [n_tt, 128, v_loc], token = t*64 + b."""
    v_loc = V // NCORES
    full = np.concatenate(
        [results[c]["out"].reshape(B * T, v_loc) for c in range(NCORES)], axis=1
    )  # [t*64+b, V]
    return np.ascontiguousarray(
        full.reshape(T, B, V).transpose(1, 0, 2)
    )


_build_cache = {}


def kernel(x, h0, c0, W_ih, W_hh, b_ih, b_hh, fc_W, fc_b, max_len):
    T = int(max_len)
    x = np.asarray(x, np.float32)
    h0 = np.asarray(h0, np.float32)
    c0 = np.asarray(c0, np.float32)
    W_ih = np.asarray(W_ih, np.float32)
    W_hh = np.asarray(W_hh, np.float32)
    b_ih = np.asarray(b_ih, np.float32)
    b_hh = np.asarray(b_hh, np.float32)
    fc_W = np.asarray(fc_W, np.float32)
    fc_b = np.asarray(fc_b, np.float32)

    if T not in _build_cache:
        _build_cache[T] = build_fused(T)
    nc = _build_cache[T]
    maps = prep_fused_inputs(
        x, h0, c0, W_ih, W_hh, b_ih, b_hh, fc_W, fc_b, T
    )
    res = run_bass_kernel_spmd(nc, maps, core_ids=list(range(NCORES)))
    return assemble_output([res.results[c] for c in range(NCORES)], T)


# revision 15
# speedup vs baseline: 1.4894x; 1.4894x over previous
"""Self-contained Trainium2 Bass kernel for the bidirectional-LSTM decoder
(nn_Decoder): 1-layer 2-direction LSTM scan over T=100 steps with a fixed
input, followed by a 32000-way vocab projection and log_softmax, distributed
over 8 NeuronCores.

Distribution (single fused SPMD launch):
  - Scan: direction-split. Cores 0-3 compute the forward LSTM for all 64
    batch rows, cores 4-7 the backward one. Weights ship as fp8e4m3
    quarters and are reassembled on-device with AllGather; fp8 stationary
    weights halve the LDWEIGHTS bandwidth that bounds the scan.
  - Transpose-free scan layout: gates^T[4H, B] accumulated per gate-quarter
    (i/f/g/o) into its own PSUM bank so the nonlinearity of quarter q
    pipelines under the matmuls of quarter q+1.
  - LSTM outputs exchange: chunked pair-AllGather (4 chunks of T/4 steps)
    overlaps the remaining scan.
  - fc + log_softmax: vocab-split (4000 rows/core). ONE matmul sweep in
    fp8 DoubleRow (K=256 per pass; o_t and fc_W both fp8), logits+bias
    staged to DRAM in bf16, softmax normalizer AllReduced in two halves so
    the fixup of half 0 (DMA+DVE: read logits, add -logZ, write output)
    overlaps the matmul sweep of half 1.
  - All recurrent state stays fp32/bf16; end-to-end max relative error vs
    the fp32 reference ~2e-3.
"""

import os
import sys

if "/opt/trn_rl_repo" not in sys.path:
    sys.path.insert(0, "/opt/trn_rl_repo")

from contextlib import ExitStack

import numpy as np

import concourse.bass as bass
import concourse.tile as tile
from concourse import mybir
from concourse.bass_utils import run_bass_kernel_spmd

F32 = mybir.dt.float32
BF16 = mybir.dt.bfloat16
FP8 = mybir.dt.float8e4
NP_BF16 = mybir.dt.np(BF16)
NP_FP8 = mybir.dt.np(FP8)
B = 64
H = 1024
V = 32000
NCORES = 8

W_SCALE = 256.0  # fp8 quant scale for W_hh / W_ih
O_SCALE = 64.0  # fp8 quant scale for LSTM outputs entering fc
FCW_SCALE = 2048.0  # fp8 quant scale for fc_W
LG_DESCALE = 1.0 / (O_SCALE * FCW_SCALE)

MAX_WAITS = 1

USE_DR = os.environ.get("BK_DR", "1") == "1"  # fp8 DoubleRow on fc matmul
N_AG_CHUNKS = int(os.environ.get("BK_AGC", "4"))  # outs AllGather chunks
N_STAT_HALVES = int(os.environ.get("BK_SH", "5"))  # stats AllReduce splits
PHASE = os.environ.get("BK_PHASE", "all")  # all | scan | fc  (timing builds only)


def split_multiwait(nc):
    """The walrus build in this environment rejects any instruction carrying
    more than one semaphore wait; hoist excess waits onto chained NOPs
    (sem-ge waits commute, so this preserves semantics)."""
    import bass_rust

    n_split = 0
    for f in nc.m.functions:
        for bb in f.blocks:
            new_insts = []
            changed = False
            for ins in bb.instructions:
                si = ins.sync_info
                if si is not None and si.on_wait and len(si.on_wait) > MAX_WAITS:
                    waits = list(si.on_wait)
                    extra, keep = waits[:-MAX_WAITS], waits[-MAX_WAITS:]
                    for j in range(0, len(extra), MAX_WAITS):
                        nop = bass_rust.InstNoOp(name=f"{ins.name}-wsplit{j}")
                        nop.engine = ins.engine
                        nop.sync_info = mybir.SyncInfo(
                            on_wait=extra[j : j + MAX_WAITS], on_update=[]
                        )
                        new_insts.append(nop)
                        n_split += 1
                    ins.sync_info = mybir.SyncInfo(
                        on_wait=keep, on_update=list(si.on_update)
                    )
                    changed = True
                new_insts.append(ins)
            if changed:
                bb.instructions = new_insts
    return n_split


def build_fused(T, v_loc=V // NCORES, timing=False):
    n_tok = B * T
    n_tt = n_tok // 128
    n_vc = v_loc // 500
    assert n_tok % 128 == 0 and v_loc % 500 == 0 and T % 2 == 0
    n_agc = N_AG_CHUNKS if T % (2 * N_AG_CHUNKS) == 0 else 1
    t_ch = T // n_agc  # steps per AllGather chunk
    n_sh = N_STAT_HALVES if n_tt % N_STAT_HALVES == 0 else 1
    tt_h = n_tt // n_sh  # token tiles per stats group

    nc = bass.Bass(num_devices=NCORES)
    whh_q = nc.declare_dram_parameter("whh_q", [2, 128, 32, 128], FP8, isOutput=False)
    wih_q = nc.declare_dram_parameter("wih_q", [2, 128, 32, 128], FP8, isOutput=False)
    xT = nc.declare_dram_parameter("xT", [128, 8, 64], BF16, isOutput=False)
    h0T = nc.declare_dram_parameter("h0T", [128, 8, 64], BF16, isOutput=False)
    c0T = nc.declare_dram_parameter("c0T", [128, 8, 64], F32, isOutput=False)
    biasT = nc.declare_dram_parameter("biasT", [128, 32], F32, isOutput=False)
    fcwT = nc.declare_dram_parameter("fcwT", [128, 16, v_loc], FP8, isOutput=False)
    fcb = nc.declare_dram_parameter("fcb", [1, v_loc], F32, isOutput=False)
    if timing:
        out = nc.dram_tensor("out", [n_tt, 128, v_loc], F32)
        chk = nc.declare_dram_parameter("chk", [128, 64], F32, isOutput=True)
    else:
        out = nc.declare_dram_parameter("out", [n_tt, 128, v_loc], F32, isOutput=True)

    ci_whh = nc.dram_tensor("ci_whh", [2, 128, 32, 128], FP8)
    co_whh = nc.dram_tensor("co_whh", [8, 128, 32, 128], FP8)
    ci_wih = nc.dram_tensor("ci_wih", [2, 128, 32, 128], FP8)
    co_wih = nc.dram_tensor("co_wih", [8, 128, 32, 128], FP8)
    outs_nat = nc.dram_tensor("outs_nat", [T, 128, 512], BF16)
    # chunk k holds [fwd steps k*t_ch..(k+1)*t_ch) ; bwd same steps]
    outs_ga = nc.dram_tensor("outs_ga", [n_agc, 2 * t_ch, 128, 512], BF16)
    lgs = nc.dram_tensor("lgs", [n_tt, 128, v_loc], BF16)
    cc_s_in = nc.dram_tensor("cc_s_in", [n_sh, 128, tt_h], F32)
    cc_s_out = nc.dram_tensor("cc_s_out", [n_sh, 128, tt_h], F32)

    ACT = mybir.ActivationFunctionType
    ALU = mybir.AluOpType
    DIR_GROUPS = [[0, 1, 2, 3], [4, 5, 6, 7]]
    PAIR_GROUPS = [[0, 4], [1, 5], [2, 6], [3, 7]]
    ALL_GROUP = [list(range(NCORES))]

    with tile.TileContext(nc) as tc, ExitStack() as ctx:
        # ---- weight gather (params -> internal -> AllGather) ----
        nc.sync.dma_start(out=ci_wih[:], in_=wih_q[:])
        nc.sync.dma_start(out=ci_whh[:], in_=whh_q[:])
        nc.gpsimd.collective_compute(
            "AllGather", ALU.bypass, replica_groups=DIR_GROUPS,
            ins=[ci_wih[:]], outs=[co_wih[:]],
        )
        nc.gpsimd.collective_compute(
            "AllGather", ALU.bypass, replica_groups=DIR_GROUPS,
            ins=[ci_whh[:]], outs=[co_whh[:]],
        )

        smalls = ctx.enter_context(tc.tile_pool(name="smalls", bufs=1))
        bias_sb = smalls.tile([128, 32], F32)
        nc.sync.dma_start(out=bias_sb, in_=biasT[:])
        h_sb = smalls.tile([128, 8, 64], BF16)
        nc.sync.dma_start(out=h_sb, in_=h0T[:])
        c_sb = smalls.tile([128, 8, 64], F32)
        nc.sync.dma_start(out=c_sb, in_=c0T[:])
        gx_sb = smalls.tile([128, 32, 64], BF16)

        # fc weights/bias: loaded up-front so the DMA hides under the scan
        fc_sing = ctx.enter_context(tc.tile_pool(name="fc_sing", bufs=1))
        w_sb = fc_sing.tile([128, 16, v_loc], FP8)
        nc.sync.dma_start(out=w_sb, in_=fcwT[:])
        bias_f32 = fc_sing.tile([128, v_loc // 8], F32)
        bias_bc = fc_sing.tile([128, v_loc], BF16)
        fcb_ap = fcb[:]
        for c8 in range(8):
            nc.sync.dma_start(
                out=bias_f32,
                in_=bass.AP(
                    tensor=fcb_ap.tensor,
                    offset=fcb_ap.offset + c8 * (v_loc // 8),
                    ap=[[0, 128], [1, v_loc // 8]],
                ),
            )
            nc.vector.tensor_copy(
                bias_bc[:, c8 * (v_loc // 8) : (c8 + 1) * (v_loc // 8)], bias_f32
            )

        # fc pools live during the scan so fc tiles interleave into it
        slab_pool = ctx.enter_context(tc.tile_pool(name="slabs", bufs=2))
        wk = ctx.enter_context(tc.tile_pool(name="fc_work", bufs=2))
        lg_pool = ctx.enter_context(tc.tile_pool(name="fc_lg", bufs=2))
        spool = ctx.enter_context(tc.tile_pool(name="fc_stats", bufs=1))
        psum_fc_cm = tc.tile_pool(name="psum_fc", bufs=1, space="PSUM")

        s_all = spool.tile([128, n_tt], F32)
        nc.vector.memset(s_all, 0.0)
        nlz = spool.tile([128, n_tt], F32)

        def pass0_tile(tt):
            o_t = slab_pool.tile([128, 16, 128], FP8 if USE_DR else BF16, tag="o_t")
            for d in range(2):
                sl = slab_pool.tile([128, 2, 512], BF16, tag=f"sl{d}")
                for t2 in range(2):
                    t = 2 * tt + t2
                    ch, ix = divmod(t, t_ch)
                    nc.sync.dma_start(
                        out=sl[:, t2, :], in_=outs_ga[ch][d * t_ch + ix]
                    )
                dst = o_t[:, d * 8 : d * 8 + 8, :].rearrange(
                    "p j (t b) -> p j t b", t=2
                )
                src = sl.rearrange("p t (j b) -> p j t b", j=8)
                if USE_DR:
                    nc.vector.tensor_scalar_mul(dst, src, O_SCALE)
                else:
                    nc.vector.tensor_copy(dst, src)
            scale = LG_DESCALE if USE_DR else (1.0 / FCW_SCALE)
            n_kp = 8 if USE_DR else 16
            for chalf in range(2):  # 4 PSUM banks per half (scan owns other 4)
                cs = range(chalf * n_vc // 2, (chalf + 1) * n_vc // 2)
                pss = {
                    c: psum_fc.tile([128, 500], F32, tag=f"ps{c % 4}", name=f"ps{c}")
                    for c in cs
                }
                for p in range(n_kp):
                    for c in cs:
                        vs = slice(c * 500, (c + 1) * 500)
                        if USE_DR:
                            nc.tensor.matmul(
                                pss[c], o_t[:, 2 * p : 2 * p + 2, :],
                                w_sb[:, 2 * p : 2 * p + 2, vs],
                                start=(p == 0), stop=(p == n_kp - 1),
                                perf_mode=mybir.MatmulPerfMode.DoubleRow,
                            )
                        else:
                            nc.tensor.matmul(
                                pss[c], o_t[:, p, :], w_sb[:, p, vs],
                                start=(p == 0), stop=(p == n_kp - 1),
                            )
                for c in cs:
                    vs = slice(c * 500, (c + 1) * 500)
                    lg = lg_pool.tile([128, 500], BF16, tag="lg")
                    nc.vector.scalar_tensor_tensor(
                        lg, pss[c], scale, bias_bc[:, vs],
                        op0=ALU.mult, op1=ALU.add,
                    )
                    ex = wk.tile([128, 500], F32, tag="ex")
                    part = wk.tile([128, 1], F32, tag="part")
                    nc.scalar.activation(ex, lg, ACT.Exp, accum_out=part)
                    nc.vector.tensor_add(
                        s_all[:, tt : tt + 1], s_all[:, tt : tt + 1], part
                    )
                    nc.sync.dma_start(out=lgs[tt][:, vs], in_=lg)

        def stats_group(h):
            ts = slice(h * tt_h, (h + 1) * tt_h)
            nc.gpsimd.dma_start(out=cc_s_in[h], in_=s_all[:, ts])
            nc.gpsimd.collective_compute(
                "AllReduce", ALU.add, replica_groups=ALL_GROUP,
                ins=[cc_s_in[h]], outs=[cc_s_out[h]],
            )
            s_glob = spool.tile([128, tt_h], F32, tag=f"sg{h}")
            nc.gpsimd.dma_start(out=s_glob, in_=cc_s_out[h])
            lnv = spool.tile([128, tt_h], F32, tag=f"ln{h}")
            nc.scalar.activation(lnv, s_glob, ACT.Ln)
            nc.vector.tensor_scalar_mul(nlz[:, ts], lnv, -1.0)

        def fixup_tile(tt):
            lg_in = fix_pool.tile([128, v_loc], BF16, tag="lgin")
            nc.sync.dma_start(out=lg_in, in_=lgs[tt])
            outt = fix_pool.tile([128, v_loc], F32, tag="outt")
            nc.vector.tensor_scalar_add(outt, lg_in, nlz[:, tt : tt + 1])
            nc.sync.dma_start(out=out[tt], in_=outt)

        do_fc = not (timing and PHASE == "scan")
        LAG = 6  # steps between an AG chunk firing and consuming its outs
        p0 = st = fx = 0

        def emit_ready_fc(t):
            """Interleave fc work for already-gathered outs into the scan."""
            nonlocal p0, st, fx
            if not do_fc:
                return
            avail = sum(1 for k in range(n_agc) if (k + 1) * t_ch - 1 + LAG <= t)
            max_tt = (t_ch * avail) // 2
            if p0 < max_tt:
                pass0_tile(p0)
                p0 += 1
            while st < n_sh and p0 >= (st + 1) * tt_h:
                stats_group(st)
                st += 1
            if fx < st * tt_h:
                fixup_tile(fx)
                fx += 1

        # ---- gx phase (wih scoped: its 32 KB frees up for fix_pool) ----
        psum_fc = ctx.enter_context(psum_fc_cm)
        with (
            tc.tile_pool(name="wih_pool", bufs=1) as wih_pool,
            tc.tile_pool(name="psum_gx", bufs=4, space="PSUM") as psum_gx,
        ):
            x_sb = smalls.tile([128, 8, 64], BF16)
            nc.sync.dma_start(out=x_sb, in_=xT[:])
            wih_sb = wih_pool.tile([128, 8, 32, 128], FP8)
            for j in range(8):
                nc.sync.dma_start(out=wih_sb[:, j, :, :], in_=co_wih[j])
            for m in range(32):
                pg = psum_gx.tile([128, 64], F32)
                for j in range(8):
                    nc.tensor.matmul(
                        pg, wih_sb[:, j, m, :], x_sb[:, j, :],
                        start=(j == 0), stop=(j == 7),
                    )
                nc.vector.tensor_scalar(
                    gx_sb[:, m, :], pg, 1.0 / W_SCALE, bias_sb[:, m : m + 1],
                    op0=ALU.mult, op1=ALU.add,
                )

        # ---- scan phase (+ interleaved fc) ----
        fix_pool = ctx.enter_context(tc.tile_pool(name="fc_fix", bufs=2))
        whh_pool = ctx.enter_context(tc.tile_pool(name="whh_pool", bufs=1))
        work = ctx.enter_context(tc.tile_pool(name="scan_work", bufs=1))
        if True:
            whh_sb = whh_pool.tile([128, 8, 32, 128], FP8)
            for j in range(8):
                nc.sync.dma_start(out=whh_sb[:, j, :, :], in_=co_whh[j])

            with tc.tile_pool(name="psum_scan", bufs=1, space="PSUM") as psum_main:
                for t in range(T):
                    nl = work.tile([128, 32, 64], BF16, tag="nl")
                    for q in range(4):  # gate quarters i, f, g, o
                        pg = psum_main.tile([128, 8, 64], F32, tag=f"pg{q}")
                        for ml in range(8):
                            m = q * 8 + ml
                            for j in range(8):
                                nc.tensor.matmul(
                                    pg[:, ml, :], whh_sb[:, j, m, :], h_sb[:, j, :],
                                    start=(j == 0), stop=(j == 7),
                                )
                        gq = work.tile([128, 8, 64], F32, tag=f"g{q}")
                        nc.vector.scalar_tensor_tensor(
                            gq, pg, 1.0 / W_SCALE, gx_sb[:, q * 8 : q * 8 + 8, :],
                            op0=ALU.mult, op1=ALU.add,
                        )
                        nc.scalar.activation(
                            nl[:, q * 8 : q * 8 + 8, :], gq,
                            ACT.Tanh if q == 2 else ACT.Sigmoid,
                        )
                    t1 = work.tile([128, 8, 64], F32, tag="t1")
                    nc.vector.tensor_mul(t1, nl[:, 0:8, :], nl[:, 16:24, :])
                    t2 = work.tile([128, 8, 64], F32, tag="t2")
                    nc.vector.tensor_mul(t2, nl[:, 8:16, :], c_sb)
                    nc.vector.tensor_add(c_sb, t1, t2)
                    tanh_c = work.tile([128, 8, 64], BF16, tag="tanh_c")
                    nc.scalar.activation(tanh_c, c_sb, ACT.Tanh)
                    nc.vector.tensor_mul(h_sb, nl[:, 24:32, :], tanh_c)
                    nc.sync.dma_start(
                        out=outs_nat[t], in_=h_sb.rearrange("p j b -> p (j b)")
                    )
                    if (t + 1) % t_ch == 0:
                        k = (t + 1) // t_ch - 1
                        nc.gpsimd.collective_compute(
                            "AllGather", ALU.bypass, replica_groups=PAIR_GROUPS,
                            ins=[outs_nat[k * t_ch : (t + 1)]], outs=[outs_ga[k]],
                        )
                    emit_ready_fc(t)

        # ---- fc drain: remaining tiles, stats, fixups ----
        if do_fc:
            while p0 < n_tt:
                pass0_tile(p0)
                p0 += 1
                while st < n_sh and p0 >= (st + 1) * tt_h:
                    stats_group(st)
                    st += 1
                if fx < st * tt_h:
                    fixup_tile(fx)
                    fx += 1
            while fx < n_tt:
                fixup_tile(fx)
                fx += 1

        if timing:
            chk_sb = spool.tile([128, 64], F32)
            if PHASE == "scan":
                nc.vector.tensor_copy(
                    chk_sb, h_sb.rearrange("p j b -> p (j b)")[:, :64]
                )
            else:
                nc.vector.tensor_copy(chk_sb[:, :n_tt], nlz)
            nc.sync.dma_start(out=chk[:, :n_tt], in_=chk_sb[:, :n_tt])

    split_multiwait(nc)
    return nc


def prep_fused_inputs(x, h0, c0, W_ih, W_hh, b_ih, b_hh, fc_W, fc_b, T):
    """Per-core in_maps. Core c: direction d = c//4, weight quarter qc = c%4,
    vocab slice c. Weights quantize to fp8e4m3 with power-of-2 scales."""
    v_loc = V // NCORES
    maps = []
    per_dir = {}
    for d in (0, 1):
        whh_full = (W_hh[d] * W_SCALE).astype(NP_FP8)
        wih_full = (W_ih[d] * W_SCALE).astype(NP_FP8)
        whh_full = whh_full.reshape(32, 128, 8, 128).transpose(3, 2, 0, 1)  # [p,j,m,q]
        wih_full = wih_full.reshape(32, 128, 8, 128).transpose(3, 2, 0, 1)
        per_dir[d] = {
            "whh": whh_full,
            "wih": wih_full,
            "h0T": np.ascontiguousarray(
                h0[d].reshape(64, 8, 128).transpose(2, 1, 0)
            ).astype(NP_BF16),
            "c0T": np.ascontiguousarray(
                c0[d].reshape(64, 8, 128).transpose(2, 1, 0)
            ).astype(np.float32),
            "biasT": np.ascontiguousarray(
                (b_ih[d] + b_hh[d]).reshape(32, 128).T
            ).astype(np.float32),
        }
    xT = np.ascontiguousarray(x.reshape(64, 8, 128).transpose(2, 1, 0)).astype(
        NP_BF16
    )
    for c in range(NCORES):
        d, qc = c // 4, c % 4
        pd = per_dir[d]
        whh_q = np.ascontiguousarray(
            pd["whh"][:, 2 * qc : 2 * qc + 2].transpose(1, 0, 2, 3)
        )
        wih_q = np.ascontiguousarray(
            pd["wih"][:, 2 * qc : 2 * qc + 2].transpose(1, 0, 2, 3)
        )
        wv = (fc_W[c * v_loc : (c + 1) * v_loc] * FCW_SCALE).astype(NP_FP8)
        fcwT = np.ascontiguousarray(wv.reshape(v_loc, 16, 128).transpose(2, 1, 0))
        maps.append(
            {
                "whh_q": whh_q,
                "wih_q": wih_q,
                "xT": xT,
                "h0T": pd["h0T"],
                "c0T": pd["c0T"],
                "biasT": pd["biasT"],
                "fcwT": fcwT,
                "fcb": np.ascontiguousarray(
                    fc_b[c * v_loc : (c + 1) * v_loc].reshape(1, v_loc)
                ).astype(np.float32),
            }
        )
    return maps


def assemble_output(results, T):
    """results[c]["out"] is [n_tt, 128, v_loc], token = t*64 + b."""
    v_loc = V // NCORES
    full = np.concatenate(
        [results[c]["out"].reshape(B * T, v_loc) for c in range(NCORES)], axis=1
    )  # [t*64+b, V]
    return np.ascontiguousarray(
        full.reshape(T, B, V).transpose(1, 0, 2)
    )


_build_cache = {}


def kernel(x, h0, c0, W_ih, W_hh, b_ih, b_hh, fc_W, fc_b, max_len):
    T = int(max_len)
    x = np.asarray(x, np.float32)
    h0 = np.asarray(h0, np.float32)
    c0 = np.asarray(c0, np.float32)
    W_ih = np.asarray(W_ih, np.float32)
    W_hh = np.asarray(W_hh, np.float32)
    b_ih = np.asarray(b_ih, np.float32)
    b_hh = np.asarray(b_hh, np.float32)
    fc_W = np.asarray(fc_W, np.float32)
    fc_b = np.asarray(fc_b, np.float32)

    if T not in _build_cache:
        _build_cache[T] = build_fused(T)
    nc = _build_cache[T]
    maps = prep_fused_inputs(
        x, h0, c0, W_ih, W_hh, b_ih, b_hh, fc_W, fc_b, T
    )
    res = run_bass_kernel_spmd(nc, maps, core_ids=list(range(NCORES)))
    return assemble_output([res.results[c] for c in range(NCORES)], T)
